# revision 8
# baseline (speedup 1.0000x reference)
"""LoRA multi-head attention on 8 Trainium2 cores.

Sharding: core c handles batch n = c//4 and head-quarter g = c%4 (4 of 16
heads, feature slice [256g, 256g+256)).  LoRA adapters and the qk scale are
folded into the projection weights on the host.

Key compaction: masked keys get softmax weight exactly 0 in the reference,
so the host gathers only the unmasked keys (padded to a multiple of 256,
padding rows forced to exp()=0 via an additive -60000 row folded into the
logits matmul as an augmented K=65 contraction row).  Each core computes the
q projection over all queries, k/v projections over the compacted keys,
blocked masked-softmax attention (exp on ACT in [128,1024] tiles, out-
matmuls of the previous block interleaved between logits matmuls to keep
PE/ACT co-busy), the unnormalized attention output plus denominator (ones
column appended to v), and a partial output projection.  Host sums partial
y over the 4 cores of each batch and reconstructs the head-averaged
attention weights from per-head unnormalized bf16 scores (scattered back to
full key positions; masked keys stay 0).
"""
import sys

sys.path.insert(0, "/opt/trn_rl_repo")

import numpy as np
import ml_dtypes

import concourse.bass as bass
import concourse.mybir as mybir
import concourse.tile as tile
from concourse import bacc
from concourse.bass_utils import run_bass_kernel_spmd
from concourse.masks import make_identity

F32 = mybir.dt.float32
F32R = mybir.dt.float32r
BF16 = mybir.dt.bfloat16

L = 2048          # sequence length (queries)
N = 2             # batch
E = 1024          # embed dim
H = 16            # total heads
HD = 64           # head dim
NH = 4            # heads per core
FS = NH * HD      # feature slice per core = 256
SCALING = 4.0
MASK_NEG = -60000.0

QT = L // 512     # 4 query tiles of 512


def _chunks(total, size):
    out, off = [], 0
    while off < total:
        out.append((off, min(size, total - off)))
        off += size
    return out


def build_nc(lkp):
    """lkp: compacted+padded key count (multiple of 256)."""
    ktk = lkp // 128   # key chunks
    pj = lkp // 256    # logits pairs per block

    nc = bacc.Bacc("TRN2", target_bir_lowering=False, debug=False)

    xT_d = nc.dram_tensor("xT", [E, L], F32, kind="ExternalInput").ap()
    xk_d = nc.dram_tensor("xk", [E, lkp], F32, kind="ExternalInput").ap()
    wqT_d = nc.dram_tensor("wqT", [E, FS], F32, kind="ExternalInput").ap()
    wkT_d = nc.dram_tensor("wkT", [E, FS], F32, kind="ExternalInput").ap()
    wvT_d = nc.dram_tensor("wvT", [E, FS], F32, kind="ExternalInput").ap()
    woT_d = nc.dram_tensor("woT", [FS, E], BF16, kind="ExternalInput").ap()
    bq_d = nc.dram_tensor("bq2", [128, 2], F32, kind="ExternalInput").ap()
    bk_d = nc.dram_tensor("bk2", [128, 2], F32, kind="ExternalInput").ap()
    bvb_d = nc.dram_tensor("bvb", [128, FS], F32, kind="ExternalInput").ap()
    mrow_d = nc.dram_tensor("mrow", [1, lkp], F32, kind="ExternalInput").ap()
    ones_d = nc.dram_tensor("onesr", [1, L], F32, kind="ExternalInput").ap()

    y_d = nc.dram_tensor("y", [L, E], F32, kind="ExternalOutput").ap()
    s_d = nc.dram_tensor("s_out", [NH, QT, pj, 128, 1024], BF16,
                         kind="ExternalOutput").ap()

    Exp = mybir.ActivationFunctionType.Exp
    Ident = mybir.ActivationFunctionType.Identity

    with tile.TileContext(nc) as tc:
        from contextlib import ExitStack

        with ExitStack() as ctx:
            persist = ctx.enter_context(tc.tile_pool(name="persist", bufs=1))
            psum_wide = ctx.enter_context(
                tc.tile_pool(name="psw", bufs=2, space="PSUM"))
            psum_acc = ctx.enter_context(
                tc.tile_pool(name="psa", bufs=4, space="PSUM"))

            # ---- persistent tiles -------------------------------------
            q_aug = [persist.tile([HD + 1, L], F32R, name=f"qaug{h}")
                     for h in range(NH)]
            k_aug = [persist.tile([HD + 1, lkp], F32R, name=f"kaug{h}")
                     for h in range(NH)]
            v_aug = [persist.tile([128, NH * (HD + 1)], BF16, name=f"vaug{t}")
                     for t in range(ktk)]
            aoT = [persist.tile([128, L], BF16, name=f"aoT{j}")
                   for j in range(2)]
            ident = persist.tile([128, 128], F32, name="ident")
            ident_b = persist.tile([128, 128], BF16, name="ident_b")
            bq_s = persist.tile([128, 2], F32, name="bq_s")
            bk_s = persist.tile([128, 2], F32, name="bk_s")
            bvb_s = persist.tile([128, FS], F32, name="bvb_s")

            make_identity(nc, ident[:])
            nc.vector.tensor_copy(ident_b[:], ident[:])
            nc.sync.dma_start(bq_s[:], bq_d[:])
            nc.sync.dma_start(bk_s[:], bk_d[:])
            nc.sync.dma_start(bvb_s[:], bvb_d[:])
            for h in range(NH):
                nc.sync.dma_start(q_aug[h][HD:HD + 1, :],
                                  ones_d[:].bitcast(F32R))
                nc.sync.dma_start(k_aug[h][HD:HD + 1, :],
                                  mrow_d[:].bitcast(F32R))

            # ---- phase A/B: load x, weights; q/k/v projections --------
            with tc.tile_pool(name="early", bufs=1) as early:
                xs = [early.tile([128, L], F32R, name=f"xs{i}")
                      for i in range(8)]
                xk = [early.tile([128, lkp], F32R, name=f"xk{i}")
                      for i in range(8)]
                wq = [early.tile([128, FS], F32R, name=f"wq{i}")
                      for i in range(8)]
                wk = [early.tile([128, FS], F32R, name=f"wk{i}")
                      for i in range(8)]
                wv = [early.tile([128, FS], F32R, name=f"wv{i}")
                      for i in range(8)]
                for i in range(8):
                    sl = slice(i * 128, (i + 1) * 128)
                    nc.sync.dma_start(xs[i][:], xT_d[sl, :].bitcast(F32R))
                    nc.sync.dma_start(xk[i][:], xk_d[sl, :].bitcast(F32R))
                    nc.sync.dma_start(wq[i][:], wqT_d[sl, :].bitcast(F32R))
                    nc.sync.dma_start(wk[i][:], wkT_d[sl, :].bitcast(F32R))
                    nc.sync.dma_start(wv[i][:], wvT_d[sl, :].bitcast(F32R))

                # q/k projections: feature-major [FS, tokens]
                for w_t, b_t, dst, x_t, ntok in (
                        (wq, bq_s, q_aug, xs, L),
                        (wk, bk_s, k_aug, xk, lkp)):
                    for ft in range(2):          # feat tile = head pair
                        for off, sz in _chunks(ntok, 512):
                            ps = psum_wide.tile([128, 512], F32, tag="wide",
                                                name="ps_p")
                            for kc in range(8):
                                nc.tensor.matmul(
                                    ps[:, 0:sz],
                                    w_t[kc][:, ft * 128:(ft + 1) * 128],
                                    x_t[kc][:, off:off + sz],
                                    start=(kc == 0), stop=(kc == 7))
                            for sub in range(2):
                                h = 2 * ft + sub
                                nc.scalar.activation(
                                    dst[h][0:HD, off:off + sz],
                                    ps[sub * 64:sub * 64 + 64, 0:sz],
                                    Ident,
                                    bias=b_t[sub * 64:sub * 64 + 64,
                                             ft:ft + 1])

                # v projection: token-major -> v_aug blocks of 65
                for tt in range(ktk):
                    ps = psum_wide.tile([128, FS], F32, tag="wide",
                                        name="ps_v")
                    for kc in range(8):
                        nc.tensor.matmul(
                            ps[:],
                            xk[kc][:, tt * 128:(tt + 1) * 128],
                            wv[kc][:],
                            start=(kc == 0), stop=(kc == 7))
                    ps3 = ps[:].rearrange("p (h d) -> p h d", h=NH)
                    bv3 = bvb_s[:].rearrange("p (h d) -> p h d", h=NH)
                    va3 = v_aug[tt][:].rearrange(
                        "p (h e) -> p h e", h=NH)[:, :, 0:HD]
                    nc.vector.tensor_add(va3, ps3, bv3)
                    nc.vector.memset(
                        v_aug[tt][:].rearrange(
                            "p (h e) -> p h e", h=NH)[:, :, HD:HD + 1],
                        1.0)

            # ---- phase C: attention -----------------------------------
            with tc.tile_pool(name="late", bufs=1) as late, \
                 tc.tile_pool(name="spool", bufs=2 * pj + 4) as spool, \
                 tc.tile_pool(name="aopool", bufs=4) as aopool, \
                 tc.tile_pool(name="rpool", bufs=4) as rpool, \
                 tc.tile_pool(name="ypool", bufs=4) as ypool, \
                 tc.tile_pool(name="stpool", bufs=3) as stpool:

                wo = [late.tile([128, E], BF16, name=f"wo{j}")
                      for j in range(2)]
                for j in range(2):
                    nc.sync.dma_start(
                        wo[j][:], woT_d[j * 128:(j + 1) * 128, :])

                blocks = [(h, qt) for h in range(NH) for qt in range(QT)]
                s_store = {}
                acc_store = {}
                NB = len(blocks)

                def emit_logits_pair(i, j):
                    h, qt = blocks[i]
                    ps_l = psum_wide.tile([128, 1024], F32, tag="wide",
                                          name="ps_l")
                    for par in range(2):
                        lk = 2 * j + par
                        nc.tensor.matmul(
                            ps_l[:, par * 512:(par + 1) * 512],
                            k_aug[h][:, lk * 128:(lk + 1) * 128],
                            q_aug[h][:, qt * 512:(qt + 1) * 512],
                            start=True, stop=True)
                    s_t = spool.tile([128, 1024], BF16, tag="s", name="s_t")
                    nc.scalar.activation(s_t[:], ps_l[:], Exp)
                    nc.sync.dma_start(s_d[h, qt, j], s_t[:])
                    s_store.setdefault(i, []).append(s_t)

                def emit_out_pair(i, j, oacc):
                    h, qt = blocks[i]
                    tiles = s_store[i]
                    for par in range(2):
                        lk = 2 * j + par
                        for sq in range(4):
                            nc.tensor.matmul(
                                oacc[sq][:],
                                tiles[j][:, par * 512 + sq * 128:
                                         par * 512 + (sq + 1) * 128],
                                v_aug[lk][:, h * (HD + 1):
                                          (h + 1) * (HD + 1)],
                                start=(lk == 0), stop=(lk == ktk - 1),
                                skip_group_check=True)

                def emit_stage(i, oacc):
                    # fast ACT evict of accumulators -> SBUF, frees PSUM
                    stg = stpool.tile([128, 4 * (HD + 1)], F32, tag="stg",
                                      name="stg")
                    for sq in range(4):
                        nc.scalar.activation(
                            stg[:, sq * (HD + 1):(sq + 1) * (HD + 1)],
                            oacc[sq][:], Ident)
                    acc_store[i] = stg

                def emit_out_tail(i):
                    h, qt = blocks[i]
                    stg = acc_store.pop(i)
                    for sq in range(4):
                        base = sq * (HD + 1)
                        recip = rpool.tile([128, 1], F32, tag="r",
                                           name="recip")
                        nc.vector.reciprocal(
                            recip[:], stg[:, base + HD:base + HD + 1])
                        ao_n = aopool.tile([128, HD], BF16, tag="ao",
                                           name="ao_n")
                        nc.vector.tensor_scalar_mul(
                            ao_n[:], stg[:, base:base + HD], recip[:])
                        ps_t = psum_acc.tile([64, 128], BF16, tag="oacc",
                                             name="ps_t")
                        nc.tensor.transpose(ps_t[:], ao_n[:], ident_b[:])
                        j2, po = h // 2, (h % 2) * 64
                        nc.vector.tensor_copy(
                            aoT[j2][po:po + 64,
                                    qt * 512 + sq * 128:
                                    qt * 512 + (sq + 1) * 128],
                            ps_t[:])

                for j in range(pj):
                    emit_logits_pair(0, j)
                for i in range(NB):
                    oacc = [psum_acc.tile([128, HD + 1], F32, tag="oacc",
                                          name=f"oacc{sq}")
                            for sq in range(4)]
                    for j in range(pj):
                        if i < NB - 1:
                            emit_logits_pair(i + 1, j)
                        emit_out_pair(i, j, oacc)
                    s_store.pop(i)
                    emit_stage(i, oacc)
                    if i > 0:
                        emit_out_tail(i - 1)
                emit_out_tail(NB - 1)

                # ---- phase D: output projection -----------------------
                for tt in range(L // 128):
                    for nh in range(2):
                        ps_y = psum_wide.tile([128, 512], F32, tag="wide",
                                              name="ps_y")
                        for j in range(2):
                            nc.tensor.matmul(
                                ps_y[:],
                                aoT[j][:, tt * 128:(tt + 1) * 128],
                                wo[j][:, nh * 512:(nh + 1) * 512],
                                start=(j == 0), stop=(j == 1))
                        y_sb = ypool.tile([128, 512], F32, tag="y",
                                          name="y_sb")
                        nc.vector.tensor_copy(y_sb[:], ps_y[:])
                        nc.sync.dma_start(
                            y_d[tt * 128:(tt + 1) * 128,
                                nh * 512:(nh + 1) * 512],
                            y_sb[:])

    nc.compile()
    return nc


_NC_CACHE = {}


def _get_nc(lkp):
    if lkp not in _NC_CACHE:
        _NC_CACHE[lkp] = build_nc(lkp)
    return _NC_CACHE[lkp]


def prepare(query, key_padding_mask, Wq, bq, Aq, Bq, Wk, bk, Ak, Bk,
            Wv, bv, Av, Bv, Wo, bo, Ao, Bo):
    """Host-side preprocessing: fold LoRA+scale, compact keys, build in_maps.

    Returns (in_maps, lkp, idx_list)."""
    query = np.asarray(query, dtype=np.float32)
    mask = np.asarray(key_padding_mask)
    scale = HD ** -0.5

    Wq_eff = (np.asarray(Wq) + SCALING * np.asarray(Bq) @ np.asarray(Aq)) * scale
    Wk_eff = np.asarray(Wk) + SCALING * np.asarray(Bk) @ np.asarray(Ak)
    Wv_eff = np.asarray(Wv) + SCALING * np.asarray(Bv) @ np.asarray(Av)
    Wo_eff = np.asarray(Wo) + SCALING * np.asarray(Bo) @ np.asarray(Ao)
    bq_eff = np.asarray(bq) * scale

    idx_list = [np.flatnonzero(~mask[n]) for n in range(N)]
    max_unmasked = max(len(ix) for ix in idx_list)
    lkp = max(256, -(-max_unmasked // 256) * 256)

    ones_r = np.ones((1, L), dtype=np.float32)
    in_maps = []
    for c in range(8):
        n, g = c // 4, c % 4
        fs = slice(FS * g, FS * g + FS)
        idx = idx_list[n]
        xT = np.ascontiguousarray(query[:, n, :].T).astype(np.float32)
        xkc = np.zeros((E, lkp), dtype=np.float32)
        xkc[:, :len(idx)] = query[idx, n, :].T
        madd = np.full((1, lkp), np.float32(MASK_NEG), dtype=np.float32)
        madd[0, :len(idx)] = 0.0
        in_maps.append({
            "xT": xT,
            "xk": xkc,
            "wqT": np.ascontiguousarray(Wq_eff[fs].T).astype(np.float32),
            "wkT": np.ascontiguousarray(Wk_eff[fs].T).astype(np.float32),
            "wvT": np.ascontiguousarray(Wv_eff[fs].T).astype(np.float32),
            "woT": np.ascontiguousarray(Wo_eff[:, fs].T).astype(
                ml_dtypes.bfloat16),
            "bq2": np.ascontiguousarray(
                bq_eff[fs].reshape(2, 128).T).astype(np.float32),
            "bk2": np.ascontiguousarray(
                np.asarray(bk)[fs].reshape(2, 128).T).astype(np.float32),
            "bvb": np.broadcast_to(np.asarray(bv)[fs], (128, FS)).astype(
                np.float32).copy(),
            "mrow": madd,
            "onesr": ones_r,
        })
    return in_maps, lkp, idx_list


def assemble(results, bo, lkp, idx_list):
    pj = lkp // 256
    bo = np.asarray(bo, dtype=np.float32)
    attn_output = np.empty((L, N, E), dtype=np.float32)
    attn_weights = np.empty((L, N, L), dtype=np.float32)
    for n in range(N):
        idx = idx_list[n]
        y = np.zeros((L, E), dtype=np.float32)
        wsum_c = np.zeros((lkp, L), dtype=np.float32)   # compacted [lk, lq]
        for g in range(4):
            r = results[4 * n + g]
            y += np.asarray(r["y"], dtype=np.float32)
            arr = np.asarray(r["s_out"])
            s = arr.reshape(NH, QT, pj, 128, 2, 512).transpose(
                0, 2, 4, 3, 1, 5).reshape(NH, lkp, L).astype(np.float32)
            for h in range(NH):
                denom = s[h].sum(axis=0)
                np.maximum(denom, np.float32(1e-37), out=denom)
                wsum_c += s[h] / denom[None, :]
        attn_output[:, n, :] = y + bo[None, :]
        wfull = np.zeros((L, L), dtype=np.float32)      # [lk, lq]
        wfull[idx, :] = wsum_c[:len(idx), :]
        attn_weights[:, n, :] = wfull.T / np.float32(H)
    return attn_output, attn_weights


def run_with(inputs, trace=False):
    in_maps, lkp, idx_list = prepare(**inputs)
    nc = _get_nc(lkp)
    res = run_bass_kernel_spmd(nc, in_maps, core_ids=list(range(8)),
                               trace=trace)
    out = assemble(res.results, inputs["bo"], lkp, idx_list)
    return out, res


def kernel(**inputs):
    out, _ = run_with(inputs, trace=False)
    return out


# revision 9
# speedup vs baseline: 1.1981x; 1.1981x over previous
"""LoRA multi-head attention on 8 Trainium2 cores.

Sharding: core c handles batch n = c//4 and head-quarter g = c%4 (4 of 16
heads, feature slice [256g, 256g+256)).  LoRA adapters and the qk scale are
folded into the projection weights on the host.

Key compaction: masked keys get softmax weight exactly 0 in the reference,
so the host gathers only the unmasked keys (padded to a multiple of 256,
padding rows forced to exp()=0 via an additive -60000 row folded into the
logits matmul as an augmented K=65 contraction row).  Each core computes the
q projection over all queries, k/v projections over the compacted keys,
blocked masked-softmax attention (exp on ACT in [128,1024] tiles, out-
matmuls of the previous block interleaved between logits matmuls to keep
PE/ACT co-busy), the unnormalized attention output plus denominator (ones
column appended to v), and a partial output projection.  Host sums partial
y over the 4 cores of each batch and reconstructs the head-averaged
attention weights from per-head unnormalized bf16 scores (scattered back to
full key positions; masked keys stay 0).
"""
import sys

sys.path.insert(0, "/opt/trn_rl_repo")

import numpy as np
import ml_dtypes

import concourse.bass as bass
import concourse.mybir as mybir
import concourse.tile as tile
from concourse import bacc
from concourse.bass_utils import run_bass_kernel_spmd
from concourse.masks import make_identity

F32 = mybir.dt.float32
F32R = mybir.dt.float32r
BF16 = mybir.dt.bfloat16

L = 2048          # sequence length (queries)
N = 2             # batch
E = 1024          # embed dim
H = 16            # total heads
HD = 64           # head dim
NH = 4            # heads per core
FS = NH * HD      # feature slice per core = 256
SCALING = 4.0
MASK_NEG = -60000.0

QT = L // 512     # 4 query tiles of 512


def _chunks(total, size):
    out, off = [], 0
    while off < total:
        out.append((off, min(size, total - off)))
        off += size
    return out


def build_nc(lkp):
    """lkp: compacted+padded key count (multiple of 256)."""
    ktk = lkp // 128   # key chunks
    pj = lkp // 256    # logits pairs per block

    nc = bacc.Bacc("TRN2", target_bir_lowering=False, debug=False)

    xT_d = nc.dram_tensor("xT", [E, L], BF16, kind="ExternalInput").ap()
    xk_d = nc.dram_tensor("xk", [E, lkp], BF16, kind="ExternalInput").ap()
    wqT_d = nc.dram_tensor("wqT", [E, FS], BF16, kind="ExternalInput").ap()
    wkT_d = nc.dram_tensor("wkT", [E, FS], BF16, kind="ExternalInput").ap()
    wvT_d = nc.dram_tensor("wvT", [E, FS], BF16, kind="ExternalInput").ap()
    woT_d = nc.dram_tensor("woT", [FS, E], BF16, kind="ExternalInput").ap()
    bq_d = nc.dram_tensor("bq2", [128, 2], F32, kind="ExternalInput").ap()
    bk_d = nc.dram_tensor("bk2", [128, 2], F32, kind="ExternalInput").ap()
    bvb_d = nc.dram_tensor("bvb", [128, FS], F32, kind="ExternalInput").ap()
    mrow_d = nc.dram_tensor("mrow", [1, lkp], BF16, kind="ExternalInput").ap()
    ones_d = nc.dram_tensor("onesr", [1, L], BF16, kind="ExternalInput").ap()

    y_d = nc.dram_tensor("y", [L, E], F32, kind="ExternalOutput").ap()
    s_d = nc.dram_tensor("s_out", [NH, QT, pj, 128, 1024], BF16,
                         kind="ExternalOutput").ap()

    Exp = mybir.ActivationFunctionType.Exp
    Ident = mybir.ActivationFunctionType.Identity

    with tile.TileContext(nc) as tc:
        from contextlib import ExitStack

        with ExitStack() as ctx:
            persist = ctx.enter_context(tc.tile_pool(name="persist", bufs=1))
            psum_wide = ctx.enter_context(
                tc.tile_pool(name="psw", bufs=2, space="PSUM"))
            psum_acc = ctx.enter_context(
                tc.tile_pool(name="psa", bufs=4, space="PSUM"))

            # ---- persistent tiles -------------------------------------
            q_aug = [persist.tile([HD + 1, L], BF16, name=f"qaug{h}")
                     for h in range(NH)]
            k_aug = [persist.tile([HD + 1, lkp], BF16, name=f"kaug{h}")
                     for h in range(NH)]
            v_aug = [persist.tile([128, NH * (HD + 1)], BF16, name=f"vaug{t}")
                     for t in range(ktk)]
            aoT = [persist.tile([128, L], BF16, name=f"aoT{j}")
                   for j in range(2)]
            ident = persist.tile([128, 128], F32, name="ident")
            ident_b = persist.tile([128, 128], BF16, name="ident_b")
            bq_s = persist.tile([128, 2], F32, name="bq_s")
            bk_s = persist.tile([128, 2], F32, name="bk_s")
            bvb_s = persist.tile([128, FS], F32, name="bvb_s")

            make_identity(nc, ident[:])
            nc.vector.tensor_copy(ident_b[:], ident[:])
            nc.sync.dma_start(bq_s[:], bq_d[:])
            nc.sync.dma_start(bk_s[:], bk_d[:])
            nc.sync.dma_start(bvb_s[:], bvb_d[:])
            for h in range(NH):
                nc.sync.dma_start(q_aug[h][HD:HD + 1, :], ones_d[:])
                nc.sync.dma_start(k_aug[h][HD:HD + 1, :], mrow_d[:])

            # ---- phase A/B: load x, weights; q/k/v projections --------
            with tc.tile_pool(name="early", bufs=1) as early:
                xs = [early.tile([128, L], BF16, name=f"xs{i}")
                      for i in range(8)]
                xk = [early.tile([128, lkp], BF16, name=f"xk{i}")
                      for i in range(8)]
                wq = [early.tile([128, FS], BF16, name=f"wq{i}")
                      for i in range(8)]
                wk = [early.tile([128, FS], BF16, name=f"wk{i}")
                      for i in range(8)]
                wv = [early.tile([128, FS], BF16, name=f"wv{i}")
                      for i in range(8)]
                for i in range(8):
                    sl = slice(i * 128, (i + 1) * 128)
                    nc.sync.dma_start(xs[i][:], xT_d[sl, :])
                    nc.sync.dma_start(xk[i][:], xk_d[sl, :])
                    nc.sync.dma_start(wq[i][:], wqT_d[sl, :])
                    nc.sync.dma_start(wk[i][:], wkT_d[sl, :])
                    nc.sync.dma_start(wv[i][:], wvT_d[sl, :])

                # q/k projections: feature-major [FS, tokens]
                for w_t, b_t, dst, x_t, ntok in (
                        (wq, bq_s, q_aug, xs, L),
                        (wk, bk_s, k_aug, xk, lkp)):
                    for ft in range(2):          # feat tile = head pair
                        for off, sz in _chunks(ntok, 512):
                            ps = psum_wide.tile([128, 512], F32, tag="wide",
                                                name="ps_p")
                            for kc in range(8):
                                nc.tensor.matmul(
                                    ps[:, 0:sz],
                                    w_t[kc][:, ft * 128:(ft + 1) * 128],
                                    x_t[kc][:, off:off + sz],
                                    start=(kc == 0), stop=(kc == 7))
                            for sub in range(2):
                                h = 2 * ft + sub
                                nc.scalar.activation(
                                    dst[h][0:HD, off:off + sz],
                                    ps[sub * 64:sub * 64 + 64, 0:sz],
                                    Ident,
                                    bias=b_t[sub * 64:sub * 64 + 64,
                                             ft:ft + 1])

                # v projection: token-major -> v_aug blocks of 65
                for tt in range(ktk):
                    ps = psum_wide.tile([128, FS], F32, tag="wide",
                                        name="ps_v")
                    for kc in range(8):
                        nc.tensor.matmul(
                            ps[:],
                            xk[kc][:, tt * 128:(tt + 1) * 128],
                            wv[kc][:],
                            start=(kc == 0), stop=(kc == 7))
                    ps3 = ps[:].rearrange("p (h d) -> p h d", h=NH)
                    bv3 = bvb_s[:].rearrange("p (h d) -> p h d", h=NH)
                    va3 = v_aug[tt][:].rearrange(
                        "p (h e) -> p h e", h=NH)[:, :, 0:HD]
                    nc.vector.tensor_add(va3, ps3, bv3)
                    nc.vector.memset(
                        v_aug[tt][:].rearrange(
                            "p (h e) -> p h e", h=NH)[:, :, HD:HD + 1],
                        1.0)

            # ---- phase C: attention -----------------------------------
            with tc.tile_pool(name="late", bufs=1) as late, \
                 tc.tile_pool(name="spool", bufs=2 * pj + 4) as spool, \
                 tc.tile_pool(name="aopool", bufs=4) as aopool, \
                 tc.tile_pool(name="rpool", bufs=4) as rpool, \
                 tc.tile_pool(name="ypool", bufs=4) as ypool, \
                 tc.tile_pool(name="stpool", bufs=3) as stpool:

                wo = [late.tile([128, E], BF16, name=f"wo{j}")
                      for j in range(2)]
                for j in range(2):
                    nc.sync.dma_start(
                        wo[j][:], woT_d[j * 128:(j + 1) * 128, :])

                blocks = [(h, qt) for h in range(NH) for qt in range(QT)]
                s_store = {}
                acc_store = {}
                NB = len(blocks)

                def emit_logits_pair(i, j):
                    h, qt = blocks[i]
                    ps_l = psum_wide.tile([128, 1024], F32, tag="wide",
                                          name="ps_l")
                    for par in range(2):
                        lk = 2 * j + par
                        nc.tensor.matmul(
                            ps_l[:, par * 512:(par + 1) * 512],
                            k_aug[h][:, lk * 128:(lk + 1) * 128],
                            q_aug[h][:, qt * 512:(qt + 1) * 512],
                            start=True, stop=True)
                    s_t = spool.tile([128, 1024], BF16, tag="s", name="s_t")
                    nc.scalar.activation(s_t[:], ps_l[:], Exp)
                    nc.sync.dma_start(s_d[h, qt, j], s_t[:])
                    s_store.setdefault(i, []).append(s_t)

                def emit_out_pair(i, j, oacc):
                    h, qt = blocks[i]
                    tiles = s_store[i]
                    for par in range(2):
                        lk = 2 * j + par
                        for sq in range(4):
                            nc.tensor.matmul(
                                oacc[sq][:],
                                tiles[j][:, par * 512 + sq * 128:
                                         par * 512 + (sq + 1) * 128],
                                v_aug[lk][:, h * (HD + 1):
                                          (h + 1) * (HD + 1)],
                                start=(lk == 0), stop=(lk == ktk - 1),
                                skip_group_check=True)

                def emit_stage(i, oacc):
                    # fast ACT evict of accumulators -> SBUF, frees PSUM
                    stg = stpool.tile([128, 4 * (HD + 1)], F32, tag="stg",
                                      name="stg")
                    for sq in range(4):
                        nc.scalar.activation(
                            stg[:, sq * (HD + 1):(sq + 1) * (HD + 1)],
                            oacc[sq][:], Ident)
                    acc_store[i] = stg

                def emit_out_tail(i):
                    h, qt = blocks[i]
                    stg = acc_store.pop(i)
                    for sq in range(4):
                        base = sq * (HD + 1)
                        recip = rpool.tile([128, 1], F32, tag="r",
                                           name="recip")
                        nc.vector.reciprocal(
                            recip[:], stg[:, base + HD:base + HD + 1])
                        ao_n = aopool.tile([128, HD], BF16, tag="ao",
                                           name="ao_n")
                        nc.vector.tensor_scalar_mul(
                            ao_n[:], stg[:, base:base + HD], recip[:])
                        ps_t = psum_acc.tile([64, 128], BF16, tag="oacc",
                                             name="ps_t")
                        nc.tensor.transpose(ps_t[:], ao_n[:], ident_b[:])
                        j2, po = h // 2, (h % 2) * 64
                        nc.vector.tensor_copy(
                            aoT[j2][po:po + 64,
                                    qt * 512 + sq * 128:
                                    qt * 512 + (sq + 1) * 128],
                            ps_t[:])

                for j in range(pj):
                    emit_logits_pair(0, j)
                for i in range(NB):
                    oacc = [psum_acc.tile([128, HD + 1], F32, tag="oacc",
                                          name=f"oacc{sq}")
                            for sq in range(4)]
                    for j in range(pj):
                        if i < NB - 1:
                            emit_logits_pair(i + 1, j)
                        emit_out_pair(i, j, oacc)
                    s_store.pop(i)
                    emit_stage(i, oacc)
                    if i > 0:
                        emit_out_tail(i - 1)
                emit_out_tail(NB - 1)

                # ---- phase D: output projection -----------------------
                for tt in range(L // 128):
                    for nh in range(2):
                        ps_y = psum_wide.tile([128, 512], F32, tag="wide",
                                              name="ps_y")
                        for j in range(2):
                            nc.tensor.matmul(
                                ps_y[:],
                                aoT[j][:, tt * 128:(tt + 1) * 128],
                                wo[j][:, nh * 512:(nh + 1) * 512],
                                start=(j == 0), stop=(j == 1))
                        y_sb = ypool.tile([128, 512], F32, tag="y",
                                          name="y_sb")
                        nc.vector.tensor_copy(y_sb[:], ps_y[:])
                        nc.sync.dma_start(
                            y_d[tt * 128:(tt + 1) * 128,
                                nh * 512:(nh + 1) * 512],
                            y_sb[:])

    nc.compile()
    return nc


_NC_CACHE = {}


def _get_nc(lkp):
    if lkp not in _NC_CACHE:
        _NC_CACHE[lkp] = build_nc(lkp)
    return _NC_CACHE[lkp]


def prepare(query, key_padding_mask, Wq, bq, Aq, Bq, Wk, bk, Ak, Bk,
            Wv, bv, Av, Bv, Wo, bo, Ao, Bo):
    """Host-side preprocessing: fold LoRA+scale, compact keys, build in_maps.

    Returns (in_maps, lkp, idx_list)."""
    query = np.asarray(query, dtype=np.float32)
    mask = np.asarray(key_padding_mask)
    scale = HD ** -0.5

    Wq_eff = (np.asarray(Wq) + SCALING * np.asarray(Bq) @ np.asarray(Aq)) * scale
    Wk_eff = np.asarray(Wk) + SCALING * np.asarray(Bk) @ np.asarray(Ak)
    Wv_eff = np.asarray(Wv) + SCALING * np.asarray(Bv) @ np.asarray(Av)
    Wo_eff = np.asarray(Wo) + SCALING * np.asarray(Bo) @ np.asarray(Ao)
    bq_eff = np.asarray(bq) * scale

    idx_list = [np.flatnonzero(~mask[n]) for n in range(N)]
    max_unmasked = max(len(ix) for ix in idx_list)
    lkp = max(256, -(-max_unmasked // 256) * 256)

    ones_r = np.ones((1, L), dtype=ml_dtypes.bfloat16)
    in_maps = []
    for c in range(8):
        n, g = c // 4, c % 4
        fs = slice(FS * g, FS * g + FS)
        idx = idx_list[n]
        xT = np.ascontiguousarray(query[:, n, :].T).astype(ml_dtypes.bfloat16)
        xkc = np.zeros((E, lkp), dtype=ml_dtypes.bfloat16)
        xkc[:, :len(idx)] = query[idx, n, :].T.astype(ml_dtypes.bfloat16)
        madd = np.full((1, lkp), np.float32(MASK_NEG), dtype=np.float32)
        madd[0, :len(idx)] = 0.0
        madd = madd.astype(ml_dtypes.bfloat16)
        in_maps.append({
            "xT": xT,
            "xk": xkc,
            "wqT": np.ascontiguousarray(Wq_eff[fs].T).astype(ml_dtypes.bfloat16),
            "wkT": np.ascontiguousarray(Wk_eff[fs].T).astype(ml_dtypes.bfloat16),
            "wvT": np.ascontiguousarray(Wv_eff[fs].T).astype(ml_dtypes.bfloat16),
            "woT": np.ascontiguousarray(Wo_eff[:, fs].T).astype(
                ml_dtypes.bfloat16),
            "bq2": np.ascontiguousarray(
                bq_eff[fs].reshape(2, 128).T).astype(np.float32),
            "bk2": np.ascontiguousarray(
                np.asarray(bk)[fs].reshape(2, 128).T).astype(np.float32),
            "bvb": np.broadcast_to(np.asarray(bv)[fs], (128, FS)).astype(
                np.float32).copy(),
            "mrow": madd,
            "onesr": ones_r,
        })
    return in_maps, lkp, idx_list


def assemble(results, bo, lkp, idx_list):
    pj = lkp // 256
    bo = np.asarray(bo, dtype=np.float32)
    attn_output = np.empty((L, N, E), dtype=np.float32)
    attn_weights = np.empty((L, N, L), dtype=np.float32)
    for n in range(N):
        idx = idx_list[n]
        y = np.zeros((L, E), dtype=np.float32)
        wsum_c = np.zeros((lkp, L), dtype=np.float32)   # compacted [lk, lq]
        for g in range(4):
            r = results[4 * n + g]
            y += np.asarray(r["y"], dtype=np.float32)
            arr = np.asarray(r["s_out"])
            s = arr.reshape(NH, QT, pj, 128, 2, 512).transpose(
                0, 2, 4, 3, 1, 5).reshape(NH, lkp, L).astype(np.float32)
            for h in range(NH):
                denom = s[h].sum(axis=0)
                np.maximum(denom, np.float32(1e-37), out=denom)
                wsum_c += s[h] / denom[None, :]
        attn_output[:, n, :] = y + bo[None, :]
        wfull = np.zeros((L, L), dtype=np.float32)      # [lk, lq]
        wfull[idx, :] = wsum_c[:len(idx), :]
        attn_weights[:, n, :] = wfull.T / np.float32(H)
    return attn_output, attn_weights


def run_with(inputs, trace=False):
    in_maps, lkp, idx_list = prepare(**inputs)
    nc = _get_nc(lkp)
    res = run_bass_kernel_spmd(nc, in_maps, core_ids=list(range(8)),
                               trace=trace)
    out = assemble(res.results, inputs["bo"], lkp, idx_list)
    return out, res


def kernel(**inputs):
    out, _ = run_with(inputs, trace=False)
    return out


# revision 10
# speedup vs baseline: 1.4105x; 1.1772x over previous
"""LoRA multi-head attention on 8 Trainium2 cores.

Sharding: core c handles batch n = c//4 and head-quarter g = c%4 (4 of 16
heads, feature slice [256g, 256g+256)).  LoRA adapters and the qk scale are
folded into the projection weights on the host.

Key compaction: masked keys get softmax weight exactly 0 in the reference,
so the host gathers only the unmasked keys (padded to a multiple of 256,
padding rows forced to exp()=0 via an additive -60000 row folded into the
logits matmul as an augmented K=65 contraction row).  Each core computes the
q projection over all queries, k/v projections over the compacted keys,
blocked masked-softmax attention (exp on ACT in [128,1024] tiles, out-
matmuls of the previous block interleaved between logits matmuls to keep
PE/ACT co-busy), the unnormalized attention output plus denominator (ones
column appended to v), and a partial output projection.  Host sums partial
y over the 4 cores of each batch and reconstructs the head-averaged
attention weights from per-head unnormalized bf16 scores (scattered back to
full key positions; masked keys stay 0).
"""
import sys

sys.path.insert(0, "/opt/trn_rl_repo")

import numpy as np
import ml_dtypes

import concourse.bass as bass
import concourse.mybir as mybir
import concourse.tile as tile
from concourse import bacc
from concourse.bass_utils import run_bass_kernel_spmd
from concourse.masks import make_identity

F32 = mybir.dt.float32
F32R = mybir.dt.float32r
BF16 = mybir.dt.bfloat16

L = 2048          # sequence length (queries)
N = 2             # batch
E = 1024          # embed dim
H = 16            # total heads
HD = 64           # head dim
NH = 4            # heads per core
FS = NH * HD      # feature slice per core = 256
SCALING = 4.0
MASK_NEG = -60000.0

QT = L // 512     # 4 query tiles of 512


def _chunks(total, size):
    out, off = [], 0
    while off < total:
        out.append((off, min(size, total - off)))
        off += size
    return out


def build_nc(lkp):
    """lkp: compacted+padded key count (multiple of 256)."""
    ktk = lkp // 128   # key chunks
    pj = lkp // 256    # logits pairs per block

    nc = bacc.Bacc("TRN2", target_bir_lowering=False, debug=False)

    xT_d = nc.dram_tensor("xT", [E, L], BF16, kind="ExternalInput").ap()
    xk_d = nc.dram_tensor("xk", [E, lkp], BF16, kind="ExternalInput").ap()
    wqT_d = nc.dram_tensor("wqT", [E, FS], BF16, kind="ExternalInput").ap()
    wkT_d = nc.dram_tensor("wkT", [E, FS], BF16, kind="ExternalInput").ap()
    wvT_d = nc.dram_tensor("wvT", [E, FS], BF16, kind="ExternalInput").ap()
    woT_d = nc.dram_tensor("woT", [FS, E], BF16, kind="ExternalInput").ap()
    bq_d = nc.dram_tensor("bq2", [128, 2], F32, kind="ExternalInput").ap()
    bk_d = nc.dram_tensor("bk2", [128, 2], F32, kind="ExternalInput").ap()
    bvb_d = nc.dram_tensor("bvb", [128, FS], F32, kind="ExternalInput").ap()
    mrow_d = nc.dram_tensor("mrow", [1, lkp], BF16, kind="ExternalInput").ap()
    ones_d = nc.dram_tensor("onesr", [1, L], BF16, kind="ExternalInput").ap()

    y_d = nc.dram_tensor("y", [L, E], F32, kind="ExternalOutput").ap()
    s_d = nc.dram_tensor("s_out", [NH, QT, pj, 128, 1024], BF16,
                         kind="ExternalOutput").ap()

    Exp = mybir.ActivationFunctionType.Exp
    Ident = mybir.ActivationFunctionType.Identity

    with tile.TileContext(nc) as tc:
        from contextlib import ExitStack

        with ExitStack() as ctx:
            persist = ctx.enter_context(tc.tile_pool(name="persist", bufs=1))
            psum_wide = ctx.enter_context(
                tc.tile_pool(name="psw", bufs=2, space="PSUM"))
            psum_acc = ctx.enter_context(
                tc.tile_pool(name="psa", bufs=4, space="PSUM"))

            # ---- persistent tiles -------------------------------------
            q_aug = [persist.tile([HD + 1, L], BF16, name=f"qaug{h}")
                     for h in range(NH)]
            k_aug = [persist.tile([HD + 1, lkp], BF16, name=f"kaug{h}")
                     for h in range(NH)]
            v_aug = [persist.tile([128, NH * (HD + 1)], BF16, name=f"vaug{t}")
                     for t in range(ktk)]
            aoT = [persist.tile([128, L], BF16, name=f"aoT{j}")
                   for j in range(2)]
            ident = persist.tile([128, 128], F32, name="ident")
            ident_b = persist.tile([128, 128], BF16, name="ident_b")
            bq_s = persist.tile([128, 2], F32, name="bq_s")
            bk_s = persist.tile([128, 2], F32, name="bk_s")
            bvb_s = persist.tile([128, FS], F32, name="bvb_s")

            make_identity(nc, ident[:])
            nc.vector.tensor_copy(ident_b[:], ident[:])
            nc.sync.dma_start(bq_s[:], bq_d[:])
            nc.sync.dma_start(bk_s[:], bk_d[:])
            nc.sync.dma_start(bvb_s[:], bvb_d[:])
            for h in range(NH):
                nc.sync.dma_start(q_aug[h][HD:HD + 1, :], ones_d[:])
                nc.sync.dma_start(k_aug[h][HD:HD + 1, :], mrow_d[:])

            # ---- phase A/B: load x, weights; q/k/v projections --------
            with tc.tile_pool(name="early", bufs=1) as early:
                qch = _chunks(L, 512)
                kch = _chunks(lkp, 512)
                xs = [[early.tile([128, sz], BF16, name=f"xs{i}_{ci}",
                                  tag=f"xs{i}_{ci}")
                       for ci, (off, sz) in enumerate(qch)]
                      for i in range(8)]
                xk = [[early.tile([128, sz], BF16, name=f"xk{i}_{ci}",
                                  tag=f"xk{i}_{ci}")
                       for ci, (off, sz) in enumerate(kch)]
                      for i in range(8)]
                wq = [early.tile([128, FS], BF16, name=f"wq{i}")
                      for i in range(8)]
                wk = [early.tile([128, FS], BF16, name=f"wk{i}")
                      for i in range(8)]
                wv = [early.tile([128, FS], BF16, name=f"wv{i}")
                      for i in range(8)]
                # weights first (small), then x token-chunk-major so the
                # first projection matmuls can start early
                for i in range(8):
                    sl = slice(i * 128, (i + 1) * 128)
                    nc.sync.dma_start(wq[i][:], wqT_d[sl, :])
                    nc.sync.dma_start(wk[i][:], wkT_d[sl, :])
                    nc.sync.dma_start(wv[i][:], wvT_d[sl, :])
                for ci, (off, sz) in enumerate(qch):
                    for i in range(8):
                        sl = slice(i * 128, (i + 1) * 128)
                        nc.sync.dma_start(xs[i][ci][:],
                                          xT_d[sl, off:off + sz])
                for ci, (off, sz) in enumerate(kch):
                    for i in range(8):
                        sl = slice(i * 128, (i + 1) * 128)
                        nc.sync.dma_start(xk[i][ci][:],
                                          xk_d[sl, off:off + sz])

                # q/k projections: feature-major [FS, tokens]
                for w_t, b_t, dst, x_t, chlist in (
                        (wq, bq_s, q_aug, xs, qch),
                        (wk, bk_s, k_aug, xk, kch)):
                    for ci, (off, sz) in enumerate(chlist):
                        for ft in range(2):      # feat tile = head pair
                            ps = psum_wide.tile([128, 512], F32, tag="wide",
                                                name="ps_p")
                            for kc in range(8):
                                nc.tensor.matmul(
                                    ps[:, 0:sz],
                                    w_t[kc][:, ft * 128:(ft + 1) * 128],
                                    x_t[kc][ci][:],
                                    start=(kc == 0), stop=(kc == 7))
                            for sub in range(2):
                                h = 2 * ft + sub
                                nc.scalar.activation(
                                    dst[h][0:HD, off:off + sz],
                                    ps[sub * 64:sub * 64 + 64, 0:sz],
                                    Ident,
                                    bias=b_t[sub * 64:sub * 64 + 64,
                                             ft:ft + 1])

                # v projection: token-major -> v_aug blocks of 65
                for tt in range(ktk):
                    ci, co = (tt * 128) // 512, (tt * 128) % 512
                    ps = psum_wide.tile([128, FS], F32, tag="wide",
                                        name="ps_v")
                    for kc in range(8):
                        nc.tensor.matmul(
                            ps[:],
                            xk[kc][ci][:, co:co + 128],
                            wv[kc][:],
                            start=(kc == 0), stop=(kc == 7))
                    ps3 = ps[:].rearrange("p (h d) -> p h d", h=NH)
                    bv3 = bvb_s[:].rearrange("p (h d) -> p h d", h=NH)
                    va3 = v_aug[tt][:].rearrange(
                        "p (h e) -> p h e", h=NH)[:, :, 0:HD]
                    nc.vector.tensor_add(va3, ps3, bv3)
                    nc.vector.memset(
                        v_aug[tt][:].rearrange(
                            "p (h e) -> p h e", h=NH)[:, :, HD:HD + 1],
                        1.0)

            # ---- phase C: attention -----------------------------------
            with tc.tile_pool(name="late", bufs=1) as late, \
                 tc.tile_pool(name="spool", bufs=2 * pj + 4) as spool, \
                 tc.tile_pool(name="aopool", bufs=4) as aopool, \
                 tc.tile_pool(name="rpool", bufs=4) as rpool, \
                 tc.tile_pool(name="ypool", bufs=4) as ypool, \
                 tc.tile_pool(name="stpool", bufs=3) as stpool:

                wo = [late.tile([128, E], BF16, name=f"wo{j}")
                      for j in range(2)]
                for j in range(2):
                    nc.sync.dma_start(
                        wo[j][:], woT_d[j * 128:(j + 1) * 128, :])

                blocks = [(h, qt) for h in range(NH) for qt in range(QT)]
                s_store = {}
                acc_store = {}
                NB = len(blocks)

                def emit_logits_pair(i, j):
                    h, qt = blocks[i]
                    ps_l = psum_wide.tile([128, 1024], F32, tag="wide",
                                          name="ps_l")
                    for par in range(2):
                        lk = 2 * j + par
                        nc.tensor.matmul(
                            ps_l[:, par * 512:(par + 1) * 512],
                            k_aug[h][:, lk * 128:(lk + 1) * 128],
                            q_aug[h][:, qt * 512:(qt + 1) * 512],
                            start=True, stop=True)
                    s_t = spool.tile([128, 1024], BF16, tag="s", name="s_t")
                    nc.scalar.activation(s_t[:], ps_l[:], Exp)
                    nc.sync.dma_start(s_d[h, qt, j], s_t[:])
                    s_store.setdefault(i, []).append(s_t)

                def emit_out_pair(i, j, oacc):
                    h, qt = blocks[i]
                    tiles = s_store[i]
                    for par in range(2):
                        lk = 2 * j + par
                        for sq in range(4):
                            nc.tensor.matmul(
                                oacc[sq][:],
                                tiles[j][:, par * 512 + sq * 128:
                                         par * 512 + (sq + 1) * 128],
                                v_aug[lk][:, h * (HD + 1):
                                          (h + 1) * (HD + 1)],
                                start=(lk == 0), stop=(lk == ktk - 1),
                                skip_group_check=True)

                def emit_stage(i, oacc):
                    # fast ACT evict of accumulators -> SBUF, frees PSUM
                    stg = stpool.tile([128, 4 * (HD + 1)], F32, tag="stg",
                                      name="stg")
                    for sq in range(4):
                        nc.vector.tensor_copy(
                            stg[:, sq * (HD + 1):(sq + 1) * (HD + 1)],
                            oacc[sq][:])
                    acc_store[i] = stg

                def emit_out_tail(i):
                    h, qt = blocks[i]
                    stg = acc_store.pop(i)
                    for sq in range(4):
                        base = sq * (HD + 1)
                        recip = rpool.tile([128, 1], F32, tag="r",
                                           name="recip")
                        nc.vector.reciprocal(
                            recip[:], stg[:, base + HD:base + HD + 1])
                        ao_n = aopool.tile([128, HD], BF16, tag="ao",
                                           name="ao_n")
                        nc.vector.tensor_scalar_mul(
                            ao_n[:], stg[:, base:base + HD], recip[:])
                        ps_t = psum_acc.tile([64, 128], BF16, tag="oacc",
                                             name="ps_t")
                        nc.tensor.transpose(ps_t[:], ao_n[:], ident_b[:])
                        j2, po = h // 2, (h % 2) * 64
                        nc.vector.tensor_copy(
                            aoT[j2][po:po + 64,
                                    qt * 512 + sq * 128:
                                    qt * 512 + (sq + 1) * 128],
                            ps_t[:])

                for j in range(pj):
                    emit_logits_pair(0, j)
                for i in range(NB):
                    oacc = [psum_acc.tile([128, HD + 1], F32, tag="oacc",
                                          name=f"oacc{sq}")
                            for sq in range(4)]
                    for j in range(pj):
                        if i < NB - 1:
                            emit_logits_pair(i + 1, j)
                        emit_out_pair(i, j, oacc)
                    s_store.pop(i)
                    emit_stage(i, oacc)
                    if i > 0:
                        emit_out_tail(i - 1)
                emit_out_tail(NB - 1)

                # ---- phase D: output projection -----------------------
                for tt in range(L // 128):
                    for nh in range(2):
                        ps_y = psum_wide.tile([128, 512], F32, tag="wide",
                                              name="ps_y")
                        for j in range(2):
                            nc.tensor.matmul(
                                ps_y[:],
                                aoT[j][:, tt * 128:(tt + 1) * 128],
                                wo[j][:, nh * 512:(nh + 1) * 512],
                                start=(j == 0), stop=(j == 1))
                        y_sb = ypool.tile([128, 512], F32, tag="y",
                                          name="y_sb")
                        nc.vector.tensor_copy(y_sb[:], ps_y[:])
                        nc.sync.dma_start(
                            y_d[tt * 128:(tt + 1) * 128,
                                nh * 512:(nh + 1) * 512],
                            y_sb[:])

    nc.compile()
    return nc


_NC_CACHE = {}


def _get_nc(lkp):
    if lkp not in _NC_CACHE:
        _NC_CACHE[lkp] = build_nc(lkp)
    return _NC_CACHE[lkp]


def prepare(query, key_padding_mask, Wq, bq, Aq, Bq, Wk, bk, Ak, Bk,
            Wv, bv, Av, Bv, Wo, bo, Ao, Bo):
    """Host-side preprocessing: fold LoRA+scale, compact keys, build in_maps.

    Returns (in_maps, lkp, idx_list)."""
    query = np.asarray(query, dtype=np.float32)
    mask = np.asarray(key_padding_mask)
    scale = HD ** -0.5

    Wq_eff = (np.asarray(Wq) + SCALING * np.asarray(Bq) @ np.asarray(Aq)) * scale
    Wk_eff = np.asarray(Wk) + SCALING * np.asarray(Bk) @ np.asarray(Ak)
    Wv_eff = np.asarray(Wv) + SCALING * np.asarray(Bv) @ np.asarray(Av)
    Wo_eff = np.asarray(Wo) + SCALING * np.asarray(Bo) @ np.asarray(Ao)
    bq_eff = np.asarray(bq) * scale

    idx_list = [np.flatnonzero(~mask[n]) for n in range(N)]
    max_unmasked = max(len(ix) for ix in idx_list)
    lkp = max(256, -(-max_unmasked // 256) * 256)

    ones_r = np.ones((1, L), dtype=ml_dtypes.bfloat16)
    in_maps = []
    for c in range(8):
        n, g = c // 4, c % 4
        fs = slice(FS * g, FS * g + FS)
        idx = idx_list[n]
        xT = np.ascontiguousarray(query[:, n, :].T).astype(ml_dtypes.bfloat16)
        xkc = np.zeros((E, lkp), dtype=ml_dtypes.bfloat16)
        xkc[:, :len(idx)] = query[idx, n, :].T.astype(ml_dtypes.bfloat16)
        madd = np.full((1, lkp), np.float32(MASK_NEG), dtype=np.float32)
        madd[0, :len(idx)] = 0.0
        madd = madd.astype(ml_dtypes.bfloat16)
        in_maps.append({
            "xT": xT,
            "xk": xkc,
            "wqT": np.ascontiguousarray(Wq_eff[fs].T).astype(ml_dtypes.bfloat16),
            "wkT": np.ascontiguousarray(Wk_eff[fs].T).astype(ml_dtypes.bfloat16),
            "wvT": np.ascontiguousarray(Wv_eff[fs].T).astype(ml_dtypes.bfloat16),
            "woT": np.ascontiguousarray(Wo_eff[:, fs].T).astype(
                ml_dtypes.bfloat16),
            "bq2": np.ascontiguousarray(
                bq_eff[fs].reshape(2, 128).T).astype(np.float32),
            "bk2": np.ascontiguousarray(
                np.asarray(bk)[fs].reshape(2, 128).T).astype(np.float32),
            "bvb": np.broadcast_to(np.asarray(bv)[fs], (128, FS)).astype(
                np.float32).copy(),
            "mrow": madd,
            "onesr": ones_r,
        })
    return in_maps, lkp, idx_list


def assemble(results, bo, lkp, idx_list):
    pj = lkp // 256
    bo = np.asarray(bo, dtype=np.float32)
    attn_output = np.empty((L, N, E), dtype=np.float32)
    attn_weights = np.empty((L, N, L), dtype=np.float32)
    for n in range(N):
        idx = idx_list[n]
        y = np.zeros((L, E), dtype=np.float32)
        wsum_c = np.zeros((lkp, L), dtype=np.float32)   # compacted [lk, lq]
        for g in range(4):
            r = results[4 * n + g]
            y += np.asarray(r["y"], dtype=np.float32)
            arr = np.asarray(r["s_out"])
            s = arr.reshape(NH, QT, pj, 128, 2, 512).transpose(
                0, 2, 4, 3, 1, 5).reshape(NH, lkp, L).astype(np.float32)
            for h in range(NH):
                denom = s[h].sum(axis=0)
                np.maximum(denom, np.float32(1e-37), out=denom)
                wsum_c += s[h] / denom[None, :]
        attn_output[:, n, :] = y + bo[None, :]
        wfull = np.zeros((L, L), dtype=np.float32)      # [lk, lq]
        wfull[idx, :] = wsum_c[:len(idx), :]
        attn_weights[:, n, :] = wfull.T / np.float32(H)
    return attn_output, attn_weights


def run_with(inputs, trace=False):
    in_maps, lkp, idx_list = prepare(**inputs)
    nc = _get_nc(lkp)
    res = run_bass_kernel_spmd(nc, in_maps, core_ids=list(range(8)),
                               trace=trace)
    out = assemble(res.results, inputs["bo"], lkp, idx_list)
    return out, res


def kernel(**inputs):
    out, _ = run_with(inputs, trace=False)
    return out
